# revision 37
# baseline (speedup 1.0000x reference)
"""Trainium2 Bass kernel for nn_MlpwithSOMModule (sum-of-max hard-attention score).

Math identity used: the reference computes
    sim  = ctx @ ent^T            # [L, M] per (b, k)
    idx  = argmax_m sim
    out  = sum_l dot(ctx_l, ent_idx[l]) = sum_l sim[l, idx[l]] = sum_l max_m sim[l, m]
so no gather/argmax is needed on device - just matmul, row-max, and a sum.

Sharding: B = 8 == n_cores, so core c processes context[c] = [64, 2, 256, 768]
(its 64 (b,k) pairs are exactly batch b == c). No cross-core communication.

The kernel is DMA-bound: each core streams 100.7 MB of fp32 from HBM at a
sustained 423-426 GB/s (measured; the SWDGE/SDMA practical cap here), so the
~280 us total is first-byte latency (~9 us NEFF preamble) + 237 us of
transfer + a short compute/teardown tail. Everything else is arranged so
compute never stalls the stream:

  1. SWDGE DMA loads 2 pairs (3.1 MB) per dma_start, casting fp32 -> bf16 in
     the DMA (halves SBUF write traffic). Tokens map to partitions as
     l = 2p + r so each partition reads 6KB contiguous bursts; the token
     permutation is harmless since sum_l max_m is order-invariant.
     Keep ~33 dma ops and an 8-deep buffer ring: finer-grained loads (1
     pair), more bufs, or split ctx/ent sems all collapse the sustained
     rate from ~425 to ~315-340 GB/s (the system has two equilibria --
     descriptor-gens are WAR-gated by PE consumption, and once the PE lags,
     rings drain, completions clump, and it never recovers).
  2. PE transposes 24 [128,128] bf16 blocks per pair (ctx and ent to
     [d, token]) into bf16 PSUM banks; ACT/DVE copy 1024-wide slabs to SBUF.
  3. 12 accumulating bf16 matmuls per pair: S[l, m] (contraction d = 6x128).
  4. DVE reduce_max over m -> row maxes collected in a [128, 128] SBUF tile.
  5. One final fp32 matmul with a ones-vector sums over l (partition dim),
     then a strided DVE add folds the two l-chunks; DMA out [1, 64].
  The last pair (63) is loaded in 3 pieces at the HEAD of the stream: left
  at the tail, its packets trickled at ~10 GB/s under end-of-kernel SBUF
  pressure, stretching the kernel by ~40 us.

HAM management (PE clock gate): the PE idles at 1.2 GHz until it sees ~3.4 us
of sustained matmul activity (transposes do not count as busy), and
re-throttles after an idle window. WARMUP_MMS dummy matmuls at t=0 flip it
to 2.4 GHz during the DMA-dead head, and PAD_MMS dummies per pair shrink the
per-load-group idle so mid-kernel re-throttles are rare. Measured best here:
~280 us (vs 313 us before warmup/load-plan tuning; memory roofline ~262 us).
"""

import sys

for _p in ("/opt/trn_rl_repo", "/root/.axon_site/_ro/trn_rl_repo"):
    if _p not in sys.path:
        sys.path.insert(0, _p)

import numpy as np

B, TOPK, L, D = 8, 64, 256, 768
N_CORES = 8
PAIRS_PER_CORE = 64  # == TOPK; one batch index per core
GROUP = 2  # pairs loaded per DMA
P = 128
DCHUNKS = D // P  # 6
LCHUNKS = L // P  # 2
WARMUP_MMS = 40  # dummy matmuls at t=0 to flip the PE HAM clock-gate to 8/8
PAD_MMS = 4  # dummy matmuls per pair: pads PE-busy per 2-pair group to ~the
# 7.4 us delivery quantum so idle chunks stay under the HAM MID window

_cache = {}


def _build():
    import concourse.bass as bass
    import concourse.mybir as mybir
    from concourse import bacc
    from concourse.tile import TileContext
    from concourse.masks import make_identity

    nc = bacc.Bacc(
        "TRN2",
        target_bir_lowering=False,
        debug=False,
        num_devices=N_CORES,
    )

    x = nc.dram_tensor(
        "x", [PAIRS_PER_CORE, 2, L, D], mybir.dt.float32, kind="ExternalInput"
    ).ap()
    out = nc.dram_tensor(
        "out", [1, PAIRS_PER_CORE], mybir.dt.float32, kind="ExternalOutput"
    ).ap()

    bf16 = mybir.dt.bfloat16
    f32 = mybir.dt.float32

    # Load plan: pairs 0..55 stream through the rotating xpool (single-pair
    # loads at the head so compute starts sooner, then 2-pair loads: 3.1 MB
    # fp32 keeps DMA efficiency high with a cheap-enough Q7 descriptor-gen
    # rate). Pairs 56..62 are "filler" pairs loaded early into resident
    # tiles and interleaved into the compute order every 8 pairs: whenever
    # load completions clump and the PE would otherwise idle past the
    # ~3.4 us HAM MID window (re-throttling it to 1.2 GHz), it has a
    # guaranteed-ready filler to chew on instead. Pair 63 is loaded in 3
    # pieces at the head for the same reason.
    reg_sizes = [1, 1] + [GROUP] * 30 + [1]  # pairs 0..62
    reg_loads = []
    s0 = 0
    for n in reg_sizes:
        reg_loads.append((s0, n))
        s0 += n
    assert s0 == 63
    compute_order = list(range(64))  # 63 = piece-loaded last pair

    # DRAM view: pair pr, partition p, chunk c=(s,r), d -- token l = 2p + r,
    # so each partition reads 2 consecutive rows (6KB contiguous DMA bursts,
    # half the packet count of the row-interleaved layout). The token
    # permutation is harmless: out = sum_l max_m is invariant to the order of
    # l and m, and the permutation is identical across all d-chunks.
    xv = x.rearrange("pr s (p two) d -> pr p s two d", p=P, two=2)

    with TileContext(nc) as tc:
        with (
            tc.tile_pool(name="const", bufs=1) as cpool,
            tc.tile_pool(name="xload", bufs=8) as xpool,
            tc.tile_pool(name="xfinal", bufs=3) as xfpool,
            tc.tile_pool(name="tpose", bufs=3) as tpool,
            tc.tile_pool(name="ppose", bufs=5, space="PSUM") as ppool,
            tc.tile_pool(name="pmm", bufs=2, space="PSUM") as mpool,
            tc.tile_pool(name="pfin", bufs=1, space="PSUM") as fpool,
        ):
            ident = cpool.tile([P, P], bf16)
            ones = cpool.tile([P, 1], f32)
            # row maxes: column 2*pair+lc holds max_m S[l, m] for l-chunk lc
            RM = cpool.tile([P, 2 * PAIRS_PER_CORE], f32)

            # PE warmup: the HAM clock gate keeps the PE at 1.2 GHz until it
            # sees a ~3.4 us window of sustained matmul activity (transposes
            # don't count). The first data lands ~11-15 us in while the PE
            # sits idle, so burn that dead time on dummy matmuls to enter the
            # kernel's steady state already at 2.4 GHz. memset on DVE keeps
            # the GpSimd queue free for DMA descriptor generation.
            warm_sb = cpool.tile([P, 512], bf16)
            nc.vector.memset(warm_sb, 0.0)
            warm_ps = fpool.tile([P, 512], f32, name="warm", tag="fin")
            for _ in range(WARMUP_MMS):
                nc.tensor.matmul(
                    warm_ps, warm_sb[:, :P], warm_sb, start=True, stop=True
                )

            def emit_mm(pair, T):
                ps = mpool.tile([P, LCHUNKS, 2 * P], f32)
                for lc in range(LCHUNKS):
                    for dc in range(DCHUNKS):
                        off = (dc * 2 + lc) * P
                        nc.tensor.matmul(
                            ps[:, lc],
                            T[:, off : off + P],  # ctxT block [d, l-chunk]
                            T[:, 1536 + dc * 2 * P : 1536 + (dc + 1) * 2 * P],
                            start=(dc == 0),
                            stop=(dc == DCHUNKS - 1),
                        )
                nc.vector.reduce_max(
                    RM[:, 2 * pair : 2 * pair + 2], ps, axis=mybir.AxisListType.X
                )
                for _ in range(PAD_MMS):
                    nc.tensor.matmul(
                        warm_ps[:, :256], warm_sb[:, :P], warm_sb[:, :256],
                        start=True, stop=True,
                    )

            LAST = PAIRS_PER_CORE - 1
            xl = xv[LAST]
            XE = xfpool.tile([P, 2, D], bf16, name="XE")
            XC0 = xfpool.tile([P, 1, D], bf16, name="XC0")
            XC1 = xfpool.tile([P, 1, D], bf16, name="XC1")

            def emit_last_pair():
                # Pair 63, loaded in 3 pieces (ent, ctx l-chunk 0/1) into
                # resident tiles at the head of the kernel.
                T = tpool.tile([P, 2 * 1536], bf16, tag="T", name="Tlast")
                Tc = T[:, :1536].rearrange("p (dc two f) -> p dc two f", two=2, f=P)
                # ent blocks j=12..23 -> T[1536:3072]
                psbA = ppool.tile([P, 1024], bf16, tag="psb", name="psbA")
                for k in range(8):
                    dc, lc = divmod(k, 2)
                    nc.tensor.transpose(
                        psbA[:, k * P : (k + 1) * P],
                        XE[:, lc, dc * P : (dc + 1) * P],
                        ident,
                    )
                nc.any.tensor_copy(T[:, 1536:2560], psbA)
                psbB = ppool.tile([P, 1024], bf16, tag="psb", name="psbB")
                for k in range(4):
                    dc, lc = divmod(k + 8, 2)
                    nc.tensor.transpose(
                        psbB[:, k * P : (k + 1) * P],
                        XE[:, lc, dc * P : (dc + 1) * P],
                        ident,
                    )
                nc.any.tensor_copy(T[:, 2560:3072], psbB[:, :512])
                ps = mpool.tile([P, LCHUNKS, 2 * P], f32, tag="ps", name="ps_last")
                for lc, XC in ((0, XC0), (1, XC1)):
                    psbC = ppool.tile([P, 1024], bf16, tag="psb", name=f"psbC{lc}")
                    for dc in range(DCHUNKS):
                        nc.tensor.transpose(
                            psbC[:, dc * P : (dc + 1) * P],
                            XC[:, 0, dc * P : (dc + 1) * P],
                            ident,
                        )
                    nc.any.tensor_copy(
                        Tc[:, :, lc, :],
                        psbC[:, :768].rearrange("p (dc f) -> p dc f", f=P),
                    )
                    for dc in range(DCHUNKS):
                        off = (dc * 2 + lc) * P
                        nc.tensor.matmul(
                            ps[:, lc],
                            T[:, off : off + P],
                            T[:, 1536 + dc * 2 * P : 1536 + (dc + 1) * 2 * P],
                            start=(dc == 0),
                            stop=(dc == DCHUNKS - 1),
                        )
                nc.vector.reduce_max(
                    RM[:, 2 * LAST : 2 * LAST + 2], ps, axis=mybir.AxisListType.X
                )

            def load(p0, n, pool, tag):
                # One dma_start per load: sustained DMA bandwidth degrades
                # once the kernel issues more than ~40 SWDGE ops (finer
                # split-sem variants collapsed from 423 to ~315 GB/s
                # mid-run), so keep the op count at 33.
                Xfull = pool.tile([P, 4 * GROUP, D], bf16, tag=tag, name=tag)
                X = Xfull[:, : 4 * n, :]
                # fp32 -> bf16 cast in DMA (SWDGE)
                nc.gpsimd.dma_start(
                    X, xv[p0 : p0 + n].rearrange("n p s two d -> p (n s) (two d)")
                )
                for q in range(n):
                    src[p0 + q] = (X, q)

            # Head emission: the first 8 xpool loads (= bufs, so no WAR yet)
            # plus all resident loads, interleaved so early-needed data leads.
            # Later xpool loads are emitted from inside the compute loop right
            # after the compute that frees their buffer slot — pool-rotation
            # WAR correctness requires emission order == use order.
            src = {}
            load(*reg_loads[0], xpool, "X")
            # identity emitted after the first dma_start so the Q7/SWDGE
            # engine starts descriptor generation immediately at kernel start
            make_identity(nc, ident)
            nc.vector.memset(ones, 1.0)
            load(*reg_loads[1], xpool, "X")
            nc.gpsimd.dma_start(XE, xl[:, 1])
            nc.gpsimd.dma_start(XC0, xl[:, 0, 0:1, :])
            nc.gpsimd.dma_start(XC1, xl[:, 0, 1:2, :])
            for ld in reg_loads[2:8]:
                load(*ld, xpool, "X")
            pending = list(reg_loads[8:])

            prev = None
            for pair in compute_order:
                if pair == LAST:
                    if prev is not None:
                        emit_mm(*prev)
                        prev = None
                    emit_last_pair()
                    continue
                X, q = src[pair]
                if prev is not None:
                    emit_mm(*prev)
                # T: ctxT at [0, 1536), entT at [1536, 3072); block (t, dc, lc)
                # lives at free offset 128*(t*12 + dc*2 + lc)
                T = tpool.tile([P, 2 * 1536], bf16, tag="T", name="T")
                for jj in range(3):
                    psb = ppool.tile([P, 1024], bf16, tag="psb", name="psb")
                    for slot in range(8):
                        j = jj * 8 + slot
                        t, rem = divmod(j, 12)
                        dc, lc = divmod(rem, 2)
                        c = q * 4 + t * 2 + lc
                        nc.tensor.transpose(
                            psb[:, slot * P : (slot + 1) * P],
                            X[:, c, dc * P : (dc + 1) * P],
                            ident,
                        )
                    nc.any.tensor_copy(T[:, jj * 1024 : (jj + 1) * 1024], psb)
                prev = (pair, T)
                # after the last pair of a regular load is consumed, emit the
                # next pending load (which reuses the just-freed buffer slot)
                if pending and (pair < 2 or q == GROUP - 1 or pair == 62):
                    load(*pending.pop(0), xpool, "X")
            if prev is not None:
                emit_mm(*prev)

            # out[pair] = sum over l = sum over 128 partitions of both lc columns
            fin = fpool.tile([1, 2 * PAIRS_PER_CORE], f32, tag="fin")
            nc.tensor.matmul(fin, ones, RM, start=True, stop=True)
            fsb = cpool.tile([1, 2 * PAIRS_PER_CORE], f32)
            nc.vector.tensor_copy(fsb, fin)
            osb = cpool.tile([1, PAIRS_PER_CORE], f32)
            fsb2 = fsb.rearrange("p (n two) -> p n two", two=2)
            nc.vector.tensor_tensor(
                osb, fsb2[:, :, 0], fsb2[:, :, 1], op=mybir.AluOpType.add
            )
            nc.sync.dma_start(out, osb)

    nc.compile()
    return nc


def _get_nc():
    if "nc" not in _cache:
        _cache["nc"] = _build()
    return _cache["nc"]


def run(context, trace=False, tmpdir=None):
    from concourse import bass_utils

    nc = _get_nc()
    context = np.ascontiguousarray(np.asarray(context, dtype=np.float32))
    assert context.shape == (B, TOPK, 2, L, D), context.shape
    in_maps = [{"x": context[c]} for c in range(N_CORES)]
    res = bass_utils.run_bass_kernel_spmd(
        nc, in_maps, core_ids=list(range(N_CORES)), trace=trace, tmpdir=tmpdir
    )
    out = np.concatenate(
        [res.results[c]["out"].reshape(1, PAIRS_PER_CORE) for c in range(N_CORES)],
        axis=0,
    ).astype(np.float32)
    return out, res


def kernel(context):
    out, _ = run(context, trace=False)
    return out



# revision 38
# speedup vs baseline: 1.1161x; 1.1161x over previous
"""Trainium2 Bass kernel for nn_MlpwithSOMModule (sum-of-max hard-attention score).

Math identity used: the reference computes
    sim  = ctx @ ent^T            # [L, M] per (b, k)
    idx  = argmax_m sim
    out  = sum_l dot(ctx_l, ent_idx[l]) = sum_l sim[l, idx[l]] = sum_l max_m sim[l, m]
so no gather/argmax is needed on device - just matmul, row-max, and a sum.

Sharding: B = 8 == n_cores, so core c processes context[c] = [64, 2, 256, 768]
(its 64 (b,k) pairs are exactly batch b == c). No cross-core communication.

The kernel is DMA-bound: each core streams 100.7 MB of fp32 from HBM at a
sustained 423-426 GB/s (measured; the SWDGE/SDMA practical cap here), so the
~280 us total is first-byte latency (~9 us NEFF preamble) + 237 us of
transfer + a short compute/teardown tail. Everything else is arranged so
compute never stalls the stream:

  1. SWDGE DMA loads 2 pairs (3.1 MB) per dma_start, casting fp32 -> bf16 in
     the DMA (halves SBUF write traffic). Tokens map to partitions as
     l = 2p + r so each partition reads 6KB contiguous bursts; the token
     permutation is harmless since sum_l max_m is order-invariant.
     Keep ~33 dma ops and an 8-deep buffer ring: finer-grained loads (1
     pair), more bufs, or split ctx/ent sems all collapse the sustained
     rate from ~425 to ~315-340 GB/s (the system has two equilibria --
     descriptor-gens are WAR-gated by PE consumption, and once the PE lags,
     rings drain, completions clump, and it never recovers).
  2. PE transposes 24 [128,128] bf16 blocks per pair (ctx and ent to
     [d, token]) into bf16 PSUM banks; ACT/DVE copy 1024-wide slabs to SBUF.
  3. 12 accumulating bf16 matmuls per pair: S[l, m] (contraction d = 6x128).
  4. DVE reduce_max over m -> row maxes collected in a [128, 128] SBUF tile.
  5. One final fp32 matmul with a ones-vector sums over l (partition dim),
     then a strided DVE add folds the two l-chunks; DMA out [1, 64].
  The last pair (63) is loaded in 3 pieces at the HEAD of the stream: left
  at the tail, its packets trickled at ~10 GB/s under end-of-kernel SBUF
  pressure, stretching the kernel by ~40 us.

HAM management (PE clock gate): the PE idles at 1.2 GHz until it sees ~3.4 us
of sustained matmul activity (transposes do not count as busy), and
re-throttles after an idle window. WARMUP_MMS dummy matmuls at t=0 flip it
to 2.4 GHz during the DMA-dead head, and PAD_MMS dummies per pair shrink the
per-load-group idle so mid-kernel re-throttles are rare. Measured best here:
~280 us (vs 313 us before warmup/load-plan tuning; memory roofline ~262 us).
"""

import sys

for _p in ("/opt/trn_rl_repo", "/root/.axon_site/_ro/trn_rl_repo"):
    if _p not in sys.path:
        sys.path.insert(0, _p)

import numpy as np

B, TOPK, L, D = 8, 64, 256, 768
N_CORES = 8
PAIRS_PER_CORE = 64  # == TOPK; one batch index per core
GROUP = 2  # pairs loaded per DMA
P = 128
DCHUNKS = D // P  # 6
LCHUNKS = L // P  # 2
WARMUP_MMS = 40  # dummy matmuls at t=0 to flip the PE HAM clock-gate to 8/8
PAD_MMS = 8  # dummy matmuls per pair: pads PE-busy per 2-pair group to ~the
# 7.4 us delivery quantum so idle chunks stay under the HAM MID window

_cache = {}


def _build():
    import concourse.bass as bass
    import concourse.mybir as mybir
    from concourse import bacc
    from concourse.tile import TileContext
    from concourse.masks import make_identity

    nc = bacc.Bacc(
        "TRN2",
        target_bir_lowering=False,
        debug=False,
        num_devices=N_CORES,
    )

    x = nc.dram_tensor(
        "x", [PAIRS_PER_CORE, 2, L, D], mybir.dt.float32, kind="ExternalInput"
    ).ap()
    out = nc.dram_tensor(
        "out", [1, PAIRS_PER_CORE], mybir.dt.float32, kind="ExternalOutput"
    ).ap()

    bf16 = mybir.dt.bfloat16
    f32 = mybir.dt.float32

    # Load plan: pairs 0..55 stream through the rotating xpool (single-pair
    # loads at the head so compute starts sooner, then 2-pair loads: 3.1 MB
    # fp32 keeps DMA efficiency high with a cheap-enough Q7 descriptor-gen
    # rate). Pairs 56..62 are "filler" pairs loaded early into resident
    # tiles and interleaved into the compute order every 8 pairs: whenever
    # load completions clump and the PE would otherwise idle past the
    # ~3.4 us HAM MID window (re-throttling it to 1.2 GHz), it has a
    # guaranteed-ready filler to chew on instead. Pair 63 is loaded in 3
    # pieces at the head for the same reason.
    reg_sizes = [1, 1] + [GROUP] * 30 + [1]  # pairs 0..62
    reg_loads = []
    s0 = 0
    for n in reg_sizes:
        reg_loads.append((s0, n))
        s0 += n
    assert s0 == 63
    compute_order = list(range(64))  # 63 = piece-loaded last pair

    # DRAM view: pair pr, partition p, chunk c=(s,r), d -- token l = 2p + r,
    # so each partition reads 2 consecutive rows (6KB contiguous DMA bursts,
    # half the packet count of the row-interleaved layout). The token
    # permutation is harmless: out = sum_l max_m is invariant to the order of
    # l and m, and the permutation is identical across all d-chunks.
    xv = x.rearrange("pr s (p two) d -> pr p s two d", p=P, two=2)

    with TileContext(nc) as tc:
        with (
            tc.tile_pool(name="const", bufs=1) as cpool,
            tc.tile_pool(name="xload", bufs=8) as xpool,
            tc.tile_pool(name="xfinal", bufs=3) as xfpool,
            tc.tile_pool(name="tpose", bufs=3) as tpool,
            tc.tile_pool(name="ppose", bufs=5, space="PSUM") as ppool,
            tc.tile_pool(name="pmm", bufs=2, space="PSUM") as mpool,
            tc.tile_pool(name="pfin", bufs=1, space="PSUM") as fpool,
        ):
            ident = cpool.tile([P, P], bf16)
            ones = cpool.tile([P, 1], f32)
            # row maxes: column 2*pair+lc holds max_m S[l, m] for l-chunk lc
            RM = cpool.tile([P, 2 * PAIRS_PER_CORE], f32)

            # PE warmup: the HAM clock gate keeps the PE at 1.2 GHz until it
            # sees a ~3.4 us window of sustained matmul activity (transposes
            # don't count). The first data lands ~11-15 us in while the PE
            # sits idle, so burn that dead time on dummy matmuls to enter the
            # kernel's steady state already at 2.4 GHz. memset on DVE keeps
            # the GpSimd queue free for DMA descriptor generation.
            warm_sb = cpool.tile([P, 512], bf16)
            nc.vector.memset(warm_sb, 0.0)
            warm_ps = fpool.tile([P, 512], f32, name="warm", tag="fin")
            for _ in range(WARMUP_MMS):
                nc.tensor.matmul(
                    warm_ps, warm_sb[:, :P], warm_sb, start=True, stop=True
                )

            def emit_mm(pair, T):
                ps = mpool.tile([P, LCHUNKS, 2 * P], f32)
                for lc in range(LCHUNKS):
                    for dc in range(DCHUNKS):
                        off = (dc * 2 + lc) * P
                        nc.tensor.matmul(
                            ps[:, lc],
                            T[:, off : off + P],  # ctxT block [d, l-chunk]
                            T[:, 1536 + dc * 2 * P : 1536 + (dc + 1) * 2 * P],
                            start=(dc == 0),
                            stop=(dc == DCHUNKS - 1),
                        )
                nc.vector.reduce_max(
                    RM[:, 2 * pair : 2 * pair + 2], ps, axis=mybir.AxisListType.X
                )
                for _ in range(PAD_MMS):
                    nc.tensor.matmul(
                        warm_ps[:, :256], warm_sb[:, :P], warm_sb[:, :256],
                        start=True, stop=True,
                    )

            LAST = PAIRS_PER_CORE - 1
            xl = xv[LAST]
            XE = xfpool.tile([P, 2, D], bf16, name="XE")
            XC0 = xfpool.tile([P, 1, D], bf16, name="XC0")
            XC1 = xfpool.tile([P, 1, D], bf16, name="XC1")

            def emit_last_pair():
                # Pair 63, loaded in 3 pieces (ent, ctx l-chunk 0/1) into
                # resident tiles at the head of the kernel.
                T = tpool.tile([P, 2 * 1536], bf16, tag="T", name="Tlast")
                Tc = T[:, :1536].rearrange("p (dc two f) -> p dc two f", two=2, f=P)
                # ent blocks j=12..23 -> T[1536:3072]
                psbA = ppool.tile([P, 1024], bf16, tag="psb", name="psbA")
                for k in range(8):
                    dc, lc = divmod(k, 2)
                    nc.tensor.transpose(
                        psbA[:, k * P : (k + 1) * P],
                        XE[:, lc, dc * P : (dc + 1) * P],
                        ident,
                    )
                nc.any.tensor_copy(T[:, 1536:2560], psbA)
                psbB = ppool.tile([P, 1024], bf16, tag="psb", name="psbB")
                for k in range(4):
                    dc, lc = divmod(k + 8, 2)
                    nc.tensor.transpose(
                        psbB[:, k * P : (k + 1) * P],
                        XE[:, lc, dc * P : (dc + 1) * P],
                        ident,
                    )
                nc.any.tensor_copy(T[:, 2560:3072], psbB[:, :512])
                ps = mpool.tile([P, LCHUNKS, 2 * P], f32, tag="ps", name="ps_last")
                for lc, XC in ((0, XC0), (1, XC1)):
                    psbC = ppool.tile([P, 1024], bf16, tag="psb", name=f"psbC{lc}")
                    for dc in range(DCHUNKS):
                        nc.tensor.transpose(
                            psbC[:, dc * P : (dc + 1) * P],
                            XC[:, 0, dc * P : (dc + 1) * P],
                            ident,
                        )
                    nc.any.tensor_copy(
                        Tc[:, :, lc, :],
                        psbC[:, :768].rearrange("p (dc f) -> p dc f", f=P),
                    )
                    for dc in range(DCHUNKS):
                        off = (dc * 2 + lc) * P
                        nc.tensor.matmul(
                            ps[:, lc],
                            T[:, off : off + P],
                            T[:, 1536 + dc * 2 * P : 1536 + (dc + 1) * 2 * P],
                            start=(dc == 0),
                            stop=(dc == DCHUNKS - 1),
                        )
                nc.vector.reduce_max(
                    RM[:, 2 * LAST : 2 * LAST + 2], ps, axis=mybir.AxisListType.X
                )

            def load(p0, n, pool, tag):
                # One dma_start per load: sustained DMA bandwidth degrades
                # once the kernel issues more than ~40 SWDGE ops (finer
                # split-sem variants collapsed from 423 to ~315 GB/s
                # mid-run), so keep the op count at 33.
                Xfull = pool.tile([P, 4 * GROUP, D], bf16, tag=tag, name=tag)
                X = Xfull[:, : 4 * n, :]
                # fp32 -> bf16 cast in DMA (SWDGE)
                nc.gpsimd.dma_start(
                    X, xv[p0 : p0 + n].rearrange("n p s two d -> p (n s) (two d)")
                )
                for q in range(n):
                    src[p0 + q] = (X, q)

            # Head emission: the first 8 xpool loads (= bufs, so no WAR yet)
            # plus all resident loads, interleaved so early-needed data leads.
            # Later xpool loads are emitted from inside the compute loop right
            # after the compute that frees their buffer slot — pool-rotation
            # WAR correctness requires emission order == use order.
            src = {}
            load(*reg_loads[0], xpool, "X")
            # identity emitted after the first dma_start so the Q7/SWDGE
            # engine starts descriptor generation immediately at kernel start
            make_identity(nc, ident)
            nc.vector.memset(ones, 1.0)
            load(*reg_loads[1], xpool, "X")
            nc.gpsimd.dma_start(XE, xl[:, 1])
            nc.gpsimd.dma_start(XC0, xl[:, 0, 0:1, :])
            nc.gpsimd.dma_start(XC1, xl[:, 0, 1:2, :])
            for ld in reg_loads[2:8]:
                load(*ld, xpool, "X")
            pending = list(reg_loads[8:])

            prev = None
            for pair in compute_order:
                if pair == LAST:
                    if prev is not None:
                        emit_mm(*prev)
                        prev = None
                    emit_last_pair()
                    continue
                X, q = src[pair]
                if prev is not None:
                    emit_mm(*prev)
                # T: ctxT at [0, 1536), entT at [1536, 3072); block (t, dc, lc)
                # lives at free offset 128*(t*12 + dc*2 + lc)
                T = tpool.tile([P, 2 * 1536], bf16, tag="T", name="T")
                for jj in range(3):
                    psb = ppool.tile([P, 1024], bf16, tag="psb", name="psb")
                    for slot in range(8):
                        j = jj * 8 + slot
                        t, rem = divmod(j, 12)
                        dc, lc = divmod(rem, 2)
                        c = q * 4 + t * 2 + lc
                        nc.tensor.transpose(
                            psb[:, slot * P : (slot + 1) * P],
                            X[:, c, dc * P : (dc + 1) * P],
                            ident,
                        )
                    nc.any.tensor_copy(T[:, jj * 1024 : (jj + 1) * 1024], psb)
                prev = (pair, T)
                # after the last pair of a regular load is consumed, emit the
                # next pending load (which reuses the just-freed buffer slot)
                if pending and (pair < 2 or q == GROUP - 1 or pair == 62):
                    load(*pending.pop(0), xpool, "X")
            if prev is not None:
                emit_mm(*prev)

            # out[pair] = sum over l = sum over 128 partitions of both lc columns
            fin = fpool.tile([1, 2 * PAIRS_PER_CORE], f32, tag="fin")
            nc.tensor.matmul(fin, ones, RM, start=True, stop=True)
            fsb = cpool.tile([1, 2 * PAIRS_PER_CORE], f32)
            nc.vector.tensor_copy(fsb, fin)
            osb = cpool.tile([1, PAIRS_PER_CORE], f32)
            fsb2 = fsb.rearrange("p (n two) -> p n two", two=2)
            nc.vector.tensor_tensor(
                osb, fsb2[:, :, 0], fsb2[:, :, 1], op=mybir.AluOpType.add
            )
            nc.sync.dma_start(out, osb)

    nc.compile()
    return nc


def _get_nc():
    if "nc" not in _cache:
        _cache["nc"] = _build()
    return _cache["nc"]


def run(context, trace=False, tmpdir=None):
    from concourse import bass_utils

    nc = _get_nc()
    context = np.ascontiguousarray(np.asarray(context, dtype=np.float32))
    assert context.shape == (B, TOPK, 2, L, D), context.shape
    in_maps = [{"x": context[c]} for c in range(N_CORES)]
    res = bass_utils.run_bass_kernel_spmd(
        nc, in_maps, core_ids=list(range(N_CORES)), trace=trace, tmpdir=tmpdir
    )
    out = np.concatenate(
        [res.results[c]["out"].reshape(1, PAIRS_PER_CORE) for c in range(N_CORES)],
        axis=0,
    ).astype(np.float32)
    return out, res


def kernel(context):
    out, _ = run(context, trace=False)
    return out

